# revision 1
# baseline (speedup 1.0000x reference)
"""DyGrEncoder (GatedGraphConv x3 + GRUCell + LSTM) as a Bass/Tile SPMD kernel
on 8 TRN2 NeuronCores.

Sharding: nodes row-wise across 8 cores, with a degree-balanced node
permutation that packs each 128-destination-node block's incoming-edge count
just under a multiple of 128 (minimizes gather-chunk padding). Per conv layer:
local m = h @ W, AllGather m (bf16), per-edge indirect-DMA gather of source
rows (edges sorted by destination block), weighted segment-sum via
accumulating one-hot matmuls into PSUM (fp32), GRU update in fp32. Final
single-step LSTM is node-parallel. Node tensors live on-chip transposed
[128 feat, nodes].

Precision: fp32 except the message path (m, AllGather, msg, S -> bf16) and
the LSTM hidden/cell inputs (bf16).
"""
import os
import numpy as np
import ml_dtypes

import concourse.bass as bass
import concourse.mybir as mybir
import concourse.tile as tile
from concourse import bacc
from concourse.bass_utils import run_bass_kernel_spmd

P = 128
NCORES = 8
f32 = mybir.dt.float32
bf16 = mybir.dt.bfloat16
i32 = mybir.dt.int32
AF = mybir.ActivationFunctionType
ALU = mybir.AluOpType
BF = ml_dtypes.bfloat16


# ----------------------------------------------------------------- host side

def _balance_nodes(dst, N, NL, NB):
    """Permute nodes so each of the 8*NB destination blocks holds 128 nodes
    whose total in-degree sits just under a multiple of 128. Returns newpos
    (orig id -> new id); new id = (core r, block j, slot) = r*NL + j*128 + s.

    Blocks at the same position j on all 8 cores share a chunk quota, so the
    SPMD edge-chunk structure stays uniform with near-zero padding."""
    indeg = np.bincount(dst, minlength=N).astype(np.int64)
    order = np.argsort(-indeg, kind='stable')      # high degree first
    lastw = NL - (NB - 1) * P                      # slots in last position
    tail_n = lastw * NCORES                        # lowest-degree nodes there
    NBF = NB - 1                                   # full positions
    body = order[:N - tail_n]
    tail = order[N - tail_n:]
    E_body = int(indeg[body].sum())
    total_chunks = (E_body + 127) // 128

    # choose number of "high" positions (quota q+1 chunks) vs "low" (q)
    q = total_chunks // (NBF * NCORES)             # per-block chunks target
    n_high = 0
    margin = 10
    sorted_deg = indeg[body]
    csum = np.concatenate([[0], np.cumsum(sorted_deg)])
    NBODY = len(body)
    while True:
        hi_bins = n_high * NCORES
        lo_bins = (NBF - n_high) * NCORES
        hi_nodes = hi_bins * P
        ok = True
        if hi_bins:
            t_hi = csum[hi_nodes]
            if t_hi / hi_bins > (q + 1) * P - margin:
                ok = False
        if lo_bins:
            t_lo = csum[NBODY] - csum[hi_nodes]
            if t_lo / lo_bins > q * P - margin:
                ok = False
        if ok or n_high >= NBF:
            break
        n_high += 1

    def snake(ids, nbins):
        """Deal degree-sorted ids into nbins equal-count bins, snake order."""
        k = len(ids) // nbins
        bins = [[] for _ in range(nbins)]
        pos = 0
        for rnd in range(k):
            idxs = range(nbins) if rnd % 2 == 0 else range(nbins - 1, -1, -1)
            for b in idxs:
                bins[b].append(ids[pos])
                pos += 1
        return bins

    hi_bins_n = n_high * NCORES
    hi_ids = body[:hi_bins_n * P]
    lo_ids = body[hi_bins_n * P:]
    bins = []
    if hi_bins_n:
        bins += snake(hi_ids, hi_bins_n)
    if NBF - n_high:
        bins += snake(lo_ids, (NBF - n_high) * NCORES)
    bins += snake(tail, NCORES)                    # last (partial) position

    # bin index -> (core, position): first n_high positions are "high"
    newpos = np.empty(N, dtype=np.int64)
    bi = 0
    for j in range(NB):
        for r in range(NCORES):
            ids = np.array(bins[bi])
            base = r * NL + j * P
            newpos[ids] = base + np.arange(len(ids))
            bi += 1
    return newpos


def _preprocess_edges(edge_index, edge_weight, N, NL, NB):
    """Per core: incoming edges sorted by destination block, each block padded
    to cap[j]*128 edges (cap shared across cores). [128, ncols] packing."""
    src = np.asarray(edge_index[0]).astype(np.int64)
    dst = np.asarray(edge_index[1]).astype(np.int64)
    w = np.asarray(edge_weight).astype(np.float32)

    per_core = []
    counts = np.zeros((NCORES, NB), dtype=np.int64)
    for r in range(NCORES):
        lo, hi = r * NL, (r + 1) * NL
        m = (dst >= lo) & (dst < hi)
        es, ed, ew = src[m], dst[m] - lo, w[m]
        order = np.argsort(ed, kind='stable')
        es, ed, ew = es[order], ed[order], ew[order]
        counts[r] = np.bincount(ed // 128, minlength=NB)
        per_core.append((es, ed, ew))

    cap = np.maximum(np.ceil(counts / 128).astype(np.int64).max(axis=0), 1)
    ncols = int(cap.sum())

    out = []
    for r in range(NCORES):
        es, ed, ew = per_core[r]
        src_idx = np.zeros(ncols * 128, dtype=np.int32)
        slot = np.zeros(ncols * 128, dtype=np.float32)
        wgt = np.zeros(ncols * 128, dtype=np.float32)
        pos = 0
        start = 0
        for j in range(NB):
            cnt = int(counts[r, j])
            seg = slice(start, start + cnt)
            src_idx[pos:pos + cnt] = es[seg]
            slot[pos:pos + cnt] = (ed[seg] - j * 128).astype(np.float32)
            wgt[pos:pos + cnt] = ew[seg]
            pos += int(cap[j]) * 128
            start += cnt
        out.append(dict(
            esrc=np.ascontiguousarray(src_idx.reshape(ncols, 128).T),
            eslot=np.ascontiguousarray(slot.reshape(ncols, 128).T),
            ew=np.ascontiguousarray(wgt.reshape(ncols, 128).T),
        ))
    return out, cap, ncols


def _padT(a, NLP, dt=np.float32):
    """[n, D] float -> [D, NLP] padded transpose."""
    aT = np.ascontiguousarray(np.asarray(a).T.astype(np.float32))
    out = np.zeros((aT.shape[0], NLP), dtype=np.float32)
    out[:, :aT.shape[1]] = aT
    return out.astype(dt)


# ---------------------------------------------------------------- bass build

def _build(N, D, L, NL, NB, NLP, cap, ncols):
    nc = bacc.Bacc("TRN2", target_bir_lowering=False, debug=False,
                   num_devices=NCORES)
    dp = nc.declare_dram_parameter

    hT0_in = dp("hT0", [P, NLP], f32, isOutput=False)
    HT_in = dp("HT", [P, NLP], bf16, isOutput=False)
    CT_in = dp("CT", [P, NLP], f32, isOutput=False)
    convW_in = dp("convW", [P, L * P], f32, isOutput=False)
    gWih_in = dp("gWihT", [P, 3 * P], f32, isOutput=False)
    gWhh_in = dp("gWhhT", [P, 3 * P], f32, isOutput=False)
    grub_in = dp("grub", [P, 4], f32, isOutput=False)
    lWih_in = dp("lWihT", [P, 4 * P], bf16, isOutput=False)
    lWhh_in = dp("lWhhT", [P, 4 * P], bf16, isOutput=False)
    lstmb_in = dp("lstmb", [P, 4], f32, isOutput=False)
    esrc_in = dp("esrc", [P, ncols], i32, isOutput=False)
    eslot_in = dp("eslot", [P, ncols], f32, isOutput=False)
    ew_in = dp("ew", [P, ncols], f32, isOutput=False)
    iota_in = dp("iota", [P, P], bf16, isOutput=False)
    Hout_ext = dp("HoutT", [P, NLP], f32, isOutput=True)
    Cout_ext = dp("CoutT", [P, NLP], f32, isOutput=True)

    lastw = NL - (NB - 1) * P          # rows in the last (partial) block
    chunks = [(s, min(512, NLP - s)) for s in range(0, NLP, 512)]

    with tile.TileContext(nc) as tc:
        with (
            tc.tile_pool(name="dram", bufs=1, space="DRAM") as dram,
            tc.tile_pool(name="persist", bufs=1) as pers,
            tc.tile_pool(name="msgp", bufs=8) as msgp,
            tc.tile_pool(name="sp", bufs=8) as sp,
            tc.tile_pool(name="tmp", bufs=2) as tp,
            tc.tile_pool(name="pagg", bufs=4, space="PSUM") as pagg,
            tc.tile_pool(name="pbig", bufs=4, space="PSUM") as pbig,
        ):
            # ---- persistent SBUF state
            hT = pers.tile([P, NLP], f32, name="hT")
            convW = pers.tile([P, L * P], f32, name="convW")
            gWih = pers.tile([P, 3 * P], f32, name="gWih")
            gWhh = pers.tile([P, 3 * P], f32, name="gWhh")
            grub = pers.tile([P, 4], f32, name="grub")
            lWih = pers.tile([P, 4 * P], bf16, name="lWih")
            lWhh = pers.tile([P, 4 * P], bf16, name="lWhh")
            lstmb = pers.tile([P, 4], f32, name="lstmb")
            esrc = pers.tile([P, ncols], i32, name="esrc")
            eslot = pers.tile([P, ncols], f32, name="eslot")
            ew = pers.tile([P, ncols], f32, name="ew")
            iota = pers.tile([P, P], bf16, name="iota")

            nc.sync.dma_start(hT[:], hT0_in[:])
            nc.sync.dma_start(convW[:], convW_in[:])
            nc.sync.dma_start(gWih[:], gWih_in[:])
            nc.sync.dma_start(gWhh[:], gWhh_in[:])
            nc.sync.dma_start(grub[:], grub_in[:])
            nc.sync.dma_start(lWih[:], lWih_in[:])
            nc.sync.dma_start(lWhh[:], lWhh_in[:])
            nc.sync.dma_start(lstmb[:], lstmb_in[:])
            nc.sync.dma_start(esrc[:], esrc_in[:])
            nc.sync.dma_start(eslot[:], eslot_in[:])
            nc.sync.dma_start(ew[:], ew_in[:])
            nc.sync.dma_start(iota[:], iota_in[:])

            with tc.tile_pool(name="conv", bufs=1) as convp:
                aggT = convp.tile([P, NLP], f32, name="aggT")
                m_sb = convp.tile([P, NLP], bf16, name="m_sb")

                for l in range(L):
                    # ---- 1. m_local = h @ W[l]  (node-major tiles, bf16)
                    for t in range(NB):
                        pm = pagg.tile([P, P], f32, name="pm", tag="agg128")
                        nc.tensor.matmul(pm[:], lhsT=hT[:, t * P:(t + 1) * P],
                                         rhs=convW[:, l * P:(l + 1) * P],
                                         start=True, stop=True)
                        nc.scalar.copy(out=m_sb[:, t * P:(t + 1) * P],
                                       in_=pm[:])

                    # ---- 2. DMA to bounce (node-major [NL, D]) + AllGather
                    m_bounce = dram.tile([NL, P], bf16, name=f"mb{l}")
                    m_full = dram.tile([N, P], bf16, name=f"mf{l}",
                                       addr_space="Shared")
                    m3 = m_sb[:].rearrange("p (t f) -> p t f", f=P)
                    nc.sync.dma_start(
                        m_bounce[:(NB - 1) * P, :].rearrange(
                            "(t p) f -> p t f", p=P),
                        m3[:, :NB - 1, :])
                    nc.sync.dma_start(m_bounce[(NB - 1) * P:, :],
                                      m3[:lastw, NB - 1, :])
                    nc.gpsimd.collective_compute(
                        "AllGather", ALU.bypass,
                        replica_groups=[list(range(NCORES))],
                        ins=[m_bounce[:].opt()], outs=[m_full[:].opt()])

                    # ---- 3. gather + weighted segment-sum into aggT
                    c = 0
                    for j in range(NB):
                        pj = pagg.tile([P, P], f32, name="pj", tag="agg128")
                        kj = int(cap[j])
                        for k in range(kj):
                            msg = msgp.tile([P, P], bf16, name="msg")
                            nc.gpsimd.indirect_dma_start(
                                out=msg[:], out_offset=None, in_=m_full[:],
                                in_offset=bass.IndirectOffsetOnAxis(
                                    ap=esrc[:, c:c + 1], axis=0))
                            S = sp.tile([P, P], bf16, name="S")
                            nc.vector.tensor_scalar(
                                out=S[:], in0=iota[:],
                                scalar1=eslot[:, c:c + 1],
                                scalar2=ew[:, c:c + 1],
                                op0=ALU.is_equal, op1=ALU.mult)
                            nc.tensor.matmul(pj[:], lhsT=msg[:], rhs=S[:],
                                             start=(k == 0),
                                             stop=(k == kj - 1))
                            c += 1
                        nc.scalar.copy(out=aggT[:, j * P:(j + 1) * P],
                                       in_=pj[:])

                    # ---- 4. GRU (x = aggT, h = hT) chunk by chunk, fp32
                    for (s, wdt) in chunks:
                        sl = slice(s, s + wdt)
                        pr = pbig.tile([P, 512], f32, name="pr", tag="big")
                        pz = pbig.tile([P, 512], f32, name="pz", tag="big")
                        pin = pbig.tile([P, 512], f32, name="pin", tag="big")
                        phn = pbig.tile([P, 512], f32, name="phn", tag="big")
                        for (ps, g) in ((pr, 0), (pz, 1)):
                            gs = slice(g * P, (g + 1) * P)
                            nc.tensor.matmul(ps[:, :wdt], lhsT=gWih[:, gs],
                                             rhs=aggT[:, sl],
                                             start=True, stop=False)
                            nc.tensor.matmul(ps[:, :wdt], lhsT=gWhh[:, gs],
                                             rhs=hT[:, sl],
                                             start=False, stop=True)
                        gn = slice(2 * P, 3 * P)
                        nc.tensor.matmul(pin[:, :wdt], lhsT=gWih[:, gn],
                                         rhs=aggT[:, sl],
                                         start=True, stop=True)
                        nc.tensor.matmul(phn[:, :wdt], lhsT=gWhh[:, gn],
                                         rhs=hT[:, sl],
                                         start=True, stop=True)

                        rt = tp.tile([P, 512], f32, name="rt", tag="ew1")
                        zt = tp.tile([P, 512], f32, name="zt", tag="ew2")
                        t2 = tp.tile([P, 512], f32, name="t2", tag="ew3")
                        t3 = tp.tile([P, 512], f32, name="t3", tag="ew4")
                        nt = tp.tile([P, 512], f32, name="nt", tag="ew5")
                        dd = tp.tile([P, 512], f32, name="dd", tag="ew6")
                        ee = tp.tile([P, 512], f32, name="ee", tag="ew7")
                        nc.scalar.activation(rt[:, :wdt], pr[:, :wdt],
                                             AF.Sigmoid, bias=grub[:, 0:1])
                        nc.scalar.activation(zt[:, :wdt], pz[:, :wdt],
                                             AF.Sigmoid, bias=grub[:, 1:2])
                        nc.vector.scalar_tensor_tensor(
                            out=t2[:, :wdt], in0=phn[:, :wdt],
                            scalar=grub[:, 3:4], in1=rt[:, :wdt],
                            op0=ALU.add, op1=ALU.mult)
                        nc.vector.tensor_add(t3[:, :wdt], t2[:, :wdt],
                                             pin[:, :wdt])
                        nc.scalar.activation(nt[:, :wdt], t3[:, :wdt],
                                             AF.Tanh, bias=grub[:, 2:3])
                        nc.vector.tensor_sub(dd[:, :wdt], hT[:, sl],
                                             nt[:, :wdt])
                        nc.vector.tensor_mul(ee[:, :wdt], zt[:, :wdt],
                                             dd[:, :wdt])
                        nc.vector.tensor_add(hT[:, sl], nt[:, :wdt],
                                             ee[:, :wdt])

            # ---- LSTM (x = hT fp32 cast to bf16, hidden bf16, cell fp32)
            if True:
                for (s, wdt) in chunks:
                    sl = slice(s, s + wdt)
                    hx = tp.tile([P, 512], bf16, name="hx", tag="ewx")
                    nc.vector.tensor_copy(hx[:, :wdt], hT[:, sl])
                    ht = tp.tile([P, 512], bf16, name="htc", tag="ewhl")
                    ct = tp.tile([P, 512], f32, name="ctc", tag="ewcl")
                    nc.sync.dma_start(ht[:, :wdt], HT_in[:, sl])
                    nc.sync.dma_start(ct[:, :wdt], CT_in[:, sl])
                    pg = [pbig.tile([P, 512], f32, name=f"pl{g}", tag="big")
                          for g in range(4)]
                    for g in range(4):
                        gs = slice(g * P, (g + 1) * P)
                        nc.tensor.matmul(pg[g][:, :wdt], lhsT=lWih[:, gs],
                                         rhs=hx[:, :wdt], start=True,
                                         stop=False)
                        nc.tensor.matmul(pg[g][:, :wdt], lhsT=lWhh[:, gs],
                                         rhs=ht[:, :wdt], start=False,
                                         stop=True)
                    it = tp.tile([P, 512], f32, name="it", tag="ew1")
                    ft = tp.tile([P, 512], f32, name="ft", tag="ew2")
                    gt = tp.tile([P, 512], f32, name="gt", tag="ew3")
                    ot = tp.tile([P, 512], f32, name="ot", tag="ew4")
                    nc.scalar.activation(it[:, :wdt], pg[0][:, :wdt],
                                         AF.Sigmoid, bias=lstmb[:, 0:1])
                    nc.scalar.activation(ft[:, :wdt], pg[1][:, :wdt],
                                         AF.Sigmoid, bias=lstmb[:, 1:2])
                    nc.scalar.activation(gt[:, :wdt], pg[2][:, :wdt],
                                         AF.Tanh, bias=lstmb[:, 2:3])
                    nc.scalar.activation(ot[:, :wdt], pg[3][:, :wdt],
                                         AF.Sigmoid, bias=lstmb[:, 3:4])
                    t1 = tp.tile([P, 512], f32, name="lt1", tag="ew5")
                    t2 = tp.tile([P, 512], f32, name="lt2", tag="ew6")
                    cn = tp.tile([P, 512], f32, name="cn", tag="ew7")
                    tc_ = tp.tile([P, 512], f32, name="tcx", tag="ewt")
                    hn = tp.tile([P, 512], f32, name="hn", tag="ewh")
                    nc.vector.tensor_mul(t1[:, :wdt], ft[:, :wdt], ct[:, :wdt])
                    nc.vector.tensor_mul(t2[:, :wdt], it[:, :wdt],
                                         gt[:, :wdt])
                    nc.vector.tensor_add(cn[:, :wdt], t1[:, :wdt],
                                         t2[:, :wdt])
                    nc.scalar.activation(tc_[:, :wdt], cn[:, :wdt], AF.Tanh)
                    nc.vector.tensor_mul(hn[:, :wdt], ot[:, :wdt],
                                         tc_[:, :wdt])
                    nc.sync.dma_start(Cout_ext[:, sl], cn[:, :wdt])
                    nc.sync.dma_start(Hout_ext[:, sl], hn[:, :wdt])
    return nc


_CACHE = {}


def kernel(X, edge_index, edge_weight, H, C, conv_W,
           gru_Wih, gru_Whh, gru_bih, gru_bhh,
           lstm_Wih, lstm_Whh, lstm_bih, lstm_bhh):
    X = np.asarray(X, dtype=np.float32)
    H = np.asarray(H, dtype=np.float32)
    C = np.asarray(C, dtype=np.float32)
    conv_W = np.asarray(conv_W, dtype=np.float32)
    edge_index = np.asarray(edge_index)
    edge_weight = np.asarray(edge_weight, dtype=np.float32)

    N, D = X.shape
    L = conv_W.shape[0]
    assert D == P and N % NCORES == 0
    NL = N // NCORES
    NB = (NL + P - 1) // P
    NLP = NB * P

    src = edge_index[0].astype(np.int64)
    dst = edge_index[1].astype(np.int64)
    newpos = _balance_nodes(dst, N, NL, NB)
    perm = np.empty(N, dtype=np.int64)          # new id -> orig id
    perm[newpos] = np.arange(N)
    e_new = np.stack([newpos[src], newpos[dst]])

    edata, cap, ncols = _preprocess_edges(e_new, edge_weight, N, NL, NB)

    key = (N, D, L, ncols, tuple(cap))
    if key not in _CACHE:
        nc = _build(N, D, L, NL, NB, NLP, cap, ncols)
        nc.compile()
        _CACHE[key] = nc
    nc = _CACHE[key]

    Xp, Hp, Cp = X[perm], H[perm], C[perm]

    gWihT = np.ascontiguousarray(np.asarray(gru_Wih, np.float32).T)
    gWhhT = np.ascontiguousarray(np.asarray(gru_Whh, np.float32).T)
    lWihT = np.ascontiguousarray(
        np.asarray(lstm_Wih, np.float32).T).astype(BF)
    lWhhT = np.ascontiguousarray(
        np.asarray(lstm_Whh, np.float32).T).astype(BF)
    gb = np.asarray(gru_bih, np.float32)
    gb2 = np.asarray(gru_bhh, np.float32)
    grub = np.stack([gb[0:D] + gb2[0:D], gb[D:2 * D] + gb2[D:2 * D],
                     gb[2 * D:3 * D], gb2[2 * D:3 * D]], axis=1)
    lb = np.asarray(lstm_bih, np.float32) + np.asarray(lstm_bhh, np.float32)
    lstmb = np.stack([lb[g * D:(g + 1) * D] for g in range(4)], axis=1)
    iota = np.ascontiguousarray(np.broadcast_to(
        np.arange(P, dtype=np.float32), (P, P))).astype(BF)
    convWb = np.ascontiguousarray(
        np.concatenate([conv_W[i] for i in range(L)], axis=1))

    in_maps = []
    for r in range(NCORES):
        sl = slice(r * NL, (r + 1) * NL)
        in_maps.append(dict(
            hT0=_padT(Xp[sl], NLP),
            HT=_padT(Hp[sl], NLP, BF),
            CT=_padT(Cp[sl], NLP),
            convW=convWb, gWihT=gWihT, gWhhT=gWhhT, grub=grub,
            lWihT=lWihT, lWhhT=lWhhT, lstmb=lstmb,
            esrc=edata[r]['esrc'], eslot=edata[r]['eslot'], ew=edata[r]['ew'],
            iota=iota,
        ))

    if os.environ.get("KERNEL_SIM"):
        from concourse import bass_interp
        sim = bass_interp.MultiCoreSim(nc, NCORES)
        for r in range(NCORES):
            for k, v in in_maps[r].items():
                sim.cores[r].tensor(k)[:] = v
        sim.simulate()
        results = [{k: np.asarray(sim.cores[r].mem_tensor(k))
                    for k in ("HoutT", "CoutT")} for r in range(NCORES)]
    else:
        trace = bool(int(os.environ.get("KERNEL_TRACE", "0")))
        res = run_bass_kernel_spmd(nc, in_maps, core_ids=list(range(NCORES)),
                                   trace=trace)
        if trace:
            kernel.last_exec_time_ns = res.exec_time_ns
        results = res.results

    Hnew = np.empty((N, D), dtype=np.float32)
    Cnew = np.empty((N, D), dtype=np.float32)
    for r in range(NCORES):
        sl = slice(r * NL, (r + 1) * NL)
        Hnew[sl] = results[r]["HoutT"].T[:NL]
        Cnew[sl] = results[r]["CoutT"].T[:NL]
    # undo permutation: row i of final output = computed row newpos[i]
    Hout = Hnew[newpos]
    Cout = Cnew[newpos]
    return Hout, Hout, Cout


kernel.last_exec_time_ns = None



# revision 6
# speedup vs baseline: 1.0308x; 1.0308x over previous
"""DyGrEncoder (GatedGraphConv x3 + GRUCell + LSTM) as a Bass/Tile SPMD kernel
on 8 TRN2 NeuronCores.

Sharding: nodes row-wise across 8 cores with a degree-balanced permutation.
Per conv layer: local m = h @ W (fp32), AllGather m (bf16) via DRAM bounce,
then a batched dma_gather (custom SWDGE gather, int16 indices) pulls per-edge
source rows from m_full in 4 source-windows of N/4 rows; the weighted
segment-sum is one-hot matmuls (host-precomputed S, streamed bf16) into
[128,512] PSUM superblock tiles, merged across windows into bf16 aggT.
GRU runs in bf16 matmuls (FWL) with fp32 state; final LSTM is node-parallel.
"""
import os
import numpy as np
import ml_dtypes

import concourse.bass as bass
import concourse.mybir as mybir
import concourse.tile as tile
from concourse import bacc, library_config
from concourse.bass_utils import run_bass_kernel_spmd

P = 128
NCORES = 8
NWIN = 4     # source windows (int16 gather index limit)
GK = 16      # edge-chunks per dma_gather call
f32 = mybir.dt.float32
bf16 = mybir.dt.bfloat16
i32 = mybir.dt.int32
i16 = mybir.dt.int16
AF = mybir.ActivationFunctionType
ALU = mybir.AluOpType
BF = ml_dtypes.bfloat16


# ----------------------------------------------------------------- host side

def _balance_nodes(dst, N, NL, NB):
    """Permute nodes so each of the 8*NB destination blocks holds 128 nodes
    whose total in-degree sits just under a multiple of 128. Returns newpos
    (orig id -> new id); new id = (core r, block j, slot) = r*NL + j*128 + s.
    """
    indeg = np.bincount(dst, minlength=N).astype(np.int64)
    order = np.argsort(-indeg, kind='stable')      # high degree first
    lastw = NL - (NB - 1) * P                      # slots in last position
    tail_n = lastw * NCORES                        # lowest-degree nodes there
    NBF = NB - 1                                   # full positions
    body = order[:N - tail_n]
    tail = order[N - tail_n:]
    E_body = int(indeg[body].sum())
    total_chunks = (E_body + 127) // 128

    q = total_chunks // (NBF * NCORES)             # per-block chunks target
    n_high = 0
    margin = 10
    sorted_deg = indeg[body]
    csum = np.concatenate([[0], np.cumsum(sorted_deg)])
    NBODY = len(body)
    while True:
        hi_bins = n_high * NCORES
        lo_bins = (NBF - n_high) * NCORES
        hi_nodes = hi_bins * P
        ok = True
        if hi_bins:
            t_hi = csum[hi_nodes]
            if t_hi / hi_bins > (q + 1) * P - margin:
                ok = False
        if lo_bins:
            t_lo = csum[NBODY] - csum[hi_nodes]
            if t_lo / lo_bins > q * P - margin:
                ok = False
        if ok or n_high >= NBF:
            break
        n_high += 1

    def snake(ids, nbins):
        k = len(ids) // nbins
        bins = [[] for _ in range(nbins)]
        pos = 0
        for rnd in range(k):
            idxs = range(nbins) if rnd % 2 == 0 else range(nbins - 1, -1, -1)
            for b in idxs:
                bins[b].append(ids[pos])
                pos += 1
        return bins

    hi_bins_n = n_high * NCORES
    hi_ids = body[:hi_bins_n * P]
    lo_ids = body[hi_bins_n * P:]
    bins = []
    if hi_bins_n:
        bins += snake(hi_ids, hi_bins_n)
    if NBF - n_high:
        bins += snake(lo_ids, (NBF - n_high) * NCORES)
    bins += snake(tail, NCORES)

    newpos = np.empty(N, dtype=np.int64)
    bi = 0
    for j in range(NB):
        for r in range(NCORES):
            ids = np.array(bins[bi])
            base = r * NL + j * P
            newpos[ids] = base + np.arange(len(ids))
            bi += 1
    return newpos


def _preprocess_edges(edge_index, edge_weight, N, NL, NB):
    """Per core: incoming edges binned by (dest block j, source window w),
    sorted by source row inside each bin, padded to cap[j][w]*128 edges
    (caps shared across cores). Chunk order is window-major:
    (w, superblock, j, k). Returns per-core packed arrays + structure.
    """
    WROWS = N // NWIN
    src = np.asarray(edge_index[0]).astype(np.int64)
    dst = np.asarray(edge_index[1]).astype(np.int64)
    w_all = np.asarray(edge_weight).astype(np.float32)

    per_core = []
    counts = np.zeros((NCORES, NB, NWIN), dtype=np.int64)
    for r in range(NCORES):
        lo, hi = r * NL, (r + 1) * NL
        m = (dst >= lo) & (dst < hi)
        es, ed, ew = src[m], dst[m] - lo, w_all[m]
        wv = es // WROWS
        order = np.lexsort((es, wv, ed // P))
        es, ed, ew, wv = es[order], ed[order], ew[order], wv[order]
        np.add.at(counts[r], (ed // P, wv), 1)
        per_core.append((es, ed, ew, wv))

    cap = np.maximum(np.ceil(counts / P).astype(np.int64).max(axis=0), 1)

    # global chunk positions, window-major over superblocks
    NSB = (NB + 3) // 4
    pos = np.zeros((NB, NWIN), dtype=np.int64)
    c = 0
    wstart = []
    for w in range(NWIN):
        wstart.append(c)
        for sb in range(NSB):
            for j in range(sb * 4, min(sb * 4 + 4, NB)):
                pos[j, w] = c
                c += int(cap[j, w])
    ncols = c
    wstart.append(c)

    out = []
    for r in range(NCORES):
        es, ed, ew, wv = per_core[r]
        rows = np.zeros((ncols, P), dtype=np.int32)   # window-relative rows
        slot = np.zeros((ncols, P), dtype=np.int64)
        wgt = np.zeros((ncols, P), dtype=np.float32)
        # bin starts within the sorted arrays
        start = 0
        for j in range(NB):
            for w in range(NWIN):
                cnt = int(counts[r, j, w])
                seg = slice(start, start + cnt)
                c0 = int(pos[j, w])
                ch = np.arange(cnt) // P + c0
                sl = np.arange(cnt) % P
                rows[ch, sl] = (es[seg] - w * WROWS).astype(np.int32)
                slot[ch, sl] = ed[seg] - j * P
                wgt[ch, sl] = ew[seg]
                start += cnt
        # S: [128 edge-slot, ncols*128], S[e, c*128+s] = w at dest slot s
        S3 = np.zeros((ncols, P, P), dtype=np.float32)
        np.put_along_axis(S3, slot[:, :, None], wgt[:, :, None], axis=2)
        S_full = np.ascontiguousarray(
            S3.transpose(1, 0, 2).reshape(P, ncols * P)).astype(BF)
        out.append(dict(rows=rows, S=S_full))
    return out, cap, pos, ncols, wstart


def _pack_idx16(rows, calls):
    """rows [ncols, 128] int32 -> [128, ncols*8] int16 in the dma_gather wrap
    layout, per call: flat q = (c-c0)*128 + p -> [q%16, c0*8 + q//16]."""
    ncols = rows.shape[0]
    out = np.zeros((16, ncols * 8), dtype=np.int16)
    for (c0, c1) in calls:
        f = rows[c0:c1].reshape(-1).astype(np.int16)
        q = np.arange(len(f))
        out[q % 16, c0 * 8 + q // 16] = f
    return np.tile(out, (8, 1))


def _padT(a, NLP, dt=np.float32):
    aT = np.ascontiguousarray(np.asarray(a).T.astype(np.float32))
    out = np.zeros((aT.shape[0], NLP), dtype=np.float32)
    out[:, :aT.shape[1]] = aT
    return out.astype(dt)


# ---------------------------------------------------------------- bass build

def _build(N, D, L, NL, NB, NLP, cap, pos, ncols, wstart, calls):
    nc = bacc.Bacc("TRN2", target_bir_lowering=False, debug=False,
                   num_devices=NCORES)
    dp = nc.declare_dram_parameter
    WROWS = N // NWIN

    hT0_in = dp("hT0", [P, NLP], f32, isOutput=False)
    HT_in = dp("HT", [P, NLP], bf16, isOutput=False)
    CT_in = dp("CT", [P, NLP], f32, isOutput=False)
    convW_in = dp("convW", [P, L * P], f32, isOutput=False)
    gWih_in = dp("gWihT", [P, 3 * P], bf16, isOutput=False)
    gWhh_in = dp("gWhhT", [P, 3 * P], bf16, isOutput=False)
    grub_in = dp("grub", [P, 4], f32, isOutput=False)
    lWih_in = dp("lWihT", [P, 4 * P], bf16, isOutput=False)
    lWhh_in = dp("lWhhT", [P, 4 * P], bf16, isOutput=False)
    lstmb_in = dp("lstmb", [P, 4], f32, isOutput=False)
    esrc_in = dp("esrc16", [P, ncols * 8], i16, isOutput=False)
    S_in = dp("S", [P, ncols * P], bf16, isOutput=False)
    Hout_ext = dp("HoutT", [P, NLP], f32, isOutput=True)
    Cout_ext = dp("CoutT", [P, NLP], f32, isOutput=True)

    lastw = NL - (NB - 1) * P
    chunks = [(s, min(512, NLP - s)) for s in range(0, NLP, 512)]
    NSB = (NB + 3) // 4

    with tile.TileContext(nc) as tc:
        with (
            tc.tile_pool(name="dram", bufs=1, space="DRAM") as dram,
            tc.tile_pool(name="persist", bufs=1) as pers,
            tc.tile_pool(name="msgp", bufs=3) as msgp,
            tc.tile_pool(name="sp", bufs=3) as sp,
            tc.tile_pool(name="tmp", bufs=2) as tp,
            tc.tile_pool(name="pagg", bufs=2, space="PSUM") as pagg,
            tc.tile_pool(name="pbig", bufs=4, space="PSUM") as pbig,
        ):
            nc.gpsimd.load_library(library_config.mlp)

            hT = pers.tile([P, NLP], f32, name="hT")
            convW = pers.tile([P, L * P], f32, name="convW")
            gWih = pers.tile([P, 3 * P], bf16, name="gWih")
            gWhh = pers.tile([P, 3 * P], bf16, name="gWhh")
            grub = pers.tile([P, 4], f32, name="grub")
            lWih = pers.tile([P, 4 * P], bf16, name="lWih")
            lWhh = pers.tile([P, 4 * P], bf16, name="lWhh")
            lstmb = pers.tile([P, 4], f32, name="lstmb")
            esrc = pers.tile([P, ncols * 8], i16, name="esrc")

            nc.sync.dma_start(hT[:], hT0_in[:])
            nc.sync.dma_start(convW[:], convW_in[:])
            nc.sync.dma_start(gWih[:], gWih_in[:])
            nc.sync.dma_start(gWhh[:], gWhh_in[:])
            nc.sync.dma_start(grub[:], grub_in[:])
            nc.sync.dma_start(lWih[:], lWih_in[:])
            nc.sync.dma_start(lWhh[:], lWhh_in[:])
            nc.sync.dma_start(lstmb[:], lstmb_in[:])
            nc.sync.dma_start(esrc[:], esrc_in[:])

            with tc.tile_pool(name="conv", bufs=1) as convp:
                aggT = convp.tile([P, NLP], bf16, name="aggT")
                m_sb = convp.tile([P, NLP], bf16, name="m_sb")

                for l in range(L):
                    # ---- 1. m_local = h @ W[l]  (node-major, bf16 out)
                    for t0 in range(0, NB, 4):
                        te = min(t0 + 4, NB)
                        pm = pagg.tile([P, 512], f32, name="pm", tag="pk")
                        for t in range(t0, te):
                            o = (t - t0) * P
                            nc.tensor.matmul(
                                pm[:, o:o + P],
                                lhsT=hT[:, t * P:(t + 1) * P],
                                rhs=convW[:, l * P:(l + 1) * P],
                                start=True, stop=True)
                        nc.scalar.copy(out=m_sb[:, t0 * P:te * P],
                                       in_=pm[:, :(te - t0) * P])

                    # ---- 2. bounce (node-major [NL, D]) + AllGather
                    m_bounce = dram.tile([NL, P], bf16, name=f"mb{l}")
                    m_full = dram.tile([N, P], bf16, name=f"mf{l}",
                                       addr_space="Shared")
                    m3 = m_sb[:].rearrange("p (t f) -> p t f", f=P)
                    nc.sync.dma_start(
                        m_bounce[:(NB - 1) * P, :].rearrange(
                            "(t p) f -> p t f", p=P),
                        m3[:, :NB - 1, :])
                    nc.sync.dma_start(m_bounce[(NB - 1) * P:, :],
                                      m3[:lastw, NB - 1, :])
                    nc.gpsimd.collective_compute(
                        "AllGather", ALU.bypass,
                        replica_groups=[list(range(NCORES))],
                        ins=[m_bounce[:].opt()], outs=[m_full[:].opt()])

                    # ---- 3. batched gathers + S stream (window-major) and
                    #         one-hot matmul segment-sum into aggT
                    ctile = {}   # global chunk -> (msg tile, S tile, col)
                    for (c0, c1) in calls:
                        kk = c1 - c0
                        msgb = msgp.tile([P, GK, P], bf16, name="msgb",
                                         tag="msgb")
                        Sb = sp.tile([P, GK * P], bf16, name="Sb", tag="Sb")
                        w = 0
                        while not (wstart[w] <= c0 < wstart[w + 1]):
                            w += 1
                        nc.sync.dma_start(Sb[:, :kk * P],
                                          S_in[:, c0 * P:c1 * P])
                        nc.gpsimd.dma_gather(
                            out_ap=msgb[:, :kk, :],
                            in_ap=m_full[w * WROWS:(w + 1) * WROWS, :],
                            idxs_ap=esrc[:, c0 * 8:c1 * 8],
                            num_idxs=kk * P, num_idxs_reg=kk * P,
                            elem_size=P, single_packet=False)
                        for c in range(c0, c1):
                            ctile[c] = (msgb, Sb, c - c0)

                        # consume any (sb, w) groups fully covered so far
                        # (handled below in lockstep order)
                    # PE consumption in the same (w, sb, j, k) order:
                    for w in range(NWIN):
                        for sb in range(NSB):
                            j0, je = sb * 4, min(sb * 4 + 4, NB)
                            pj = pagg.tile([P, 512], f32, name="pj", tag="pk")
                            for j in range(j0, je):
                                kj = int(cap[j, w])
                                base = int(pos[j, w])
                                for k in range(kj):
                                    mt, St, col = ctile[base + k]
                                    nc.tensor.matmul(
                                        pj[:, (j - j0) * P:(j - j0 + 1) * P],
                                        lhsT=mt[:, col, :],
                                        rhs=St[:, col * P:(col + 1) * P],
                                        start=(k == 0), stop=(k == kj - 1))
                            sl = slice(j0 * P, je * P)
                            wdt = (je - j0) * P
                            if w == 0:
                                nc.scalar.copy(out=aggT[:, sl],
                                               in_=pj[:, :wdt])
                            else:
                                nc.vector.tensor_add(aggT[:, sl],
                                                     aggT[:, sl],
                                                     pj[:, :wdt])

                    # ---- 4. GRU (x = aggT bf16, h = hT fp32 + bf16 shadow)
                    for (s, wdt) in chunks:
                        sl = slice(s, s + wdt)
                        hb = tp.tile([P, 512], bf16, name="hb", tag="ewhb")
                        nc.vector.tensor_copy(hb[:, :wdt], hT[:, sl])
                        pr = pbig.tile([P, 512], f32, name="pr", tag="big")
                        pz = pbig.tile([P, 512], f32, name="pz", tag="big")
                        pin = pbig.tile([P, 512], f32, name="pin", tag="big")
                        phn = pbig.tile([P, 512], f32, name="phn", tag="big")
                        for (ps, g) in ((pr, 0), (pz, 1)):
                            gs = slice(g * P, (g + 1) * P)
                            nc.tensor.matmul(ps[:, :wdt], lhsT=gWih[:, gs],
                                             rhs=aggT[:, sl],
                                             start=True, stop=False)
                            nc.tensor.matmul(ps[:, :wdt], lhsT=gWhh[:, gs],
                                             rhs=hb[:, :wdt],
                                             start=False, stop=True)
                        gn = slice(2 * P, 3 * P)
                        nc.tensor.matmul(pin[:, :wdt], lhsT=gWih[:, gn],
                                         rhs=aggT[:, sl],
                                         start=True, stop=True)
                        nc.tensor.matmul(phn[:, :wdt], lhsT=gWhh[:, gn],
                                         rhs=hb[:, :wdt],
                                         start=True, stop=True)

                        rt = tp.tile([P, 512], f32, name="rt", tag="ew1")
                        zt = tp.tile([P, 512], f32, name="zt", tag="ew2")
                        t2 = tp.tile([P, 512], f32, name="t2", tag="ew3")
                        t3 = tp.tile([P, 512], f32, name="t3", tag="ew4")
                        nt = tp.tile([P, 512], f32, name="nt", tag="ew5")
                        dd = tp.tile([P, 512], f32, name="dd", tag="ew6")
                        ee = tp.tile([P, 512], f32, name="ee", tag="ew7")
                        nc.scalar.activation(rt[:, :wdt], pr[:, :wdt],
                                             AF.Sigmoid, bias=grub[:, 0:1])
                        nc.scalar.activation(zt[:, :wdt], pz[:, :wdt],
                                             AF.Sigmoid, bias=grub[:, 1:2])
                        nc.vector.scalar_tensor_tensor(
                            out=t2[:, :wdt], in0=phn[:, :wdt],
                            scalar=grub[:, 3:4], in1=rt[:, :wdt],
                            op0=ALU.add, op1=ALU.mult)
                        nc.vector.tensor_add(t3[:, :wdt], t2[:, :wdt],
                                             pin[:, :wdt])
                        nc.scalar.activation(nt[:, :wdt], t3[:, :wdt],
                                             AF.Tanh, bias=grub[:, 2:3])
                        nc.vector.tensor_sub(dd[:, :wdt], hT[:, sl],
                                             nt[:, :wdt])
                        nc.vector.tensor_mul(ee[:, :wdt], zt[:, :wdt],
                                             dd[:, :wdt])
                        nc.vector.tensor_add(hT[:, sl], nt[:, :wdt],
                                             ee[:, :wdt])

            # ---- LSTM (x = hT cast bf16, hidden bf16, cell fp32)
            for (s, wdt) in chunks:
                sl = slice(s, s + wdt)
                hx = tp.tile([P, 512], bf16, name="hx", tag="ewhb")
                nc.vector.tensor_copy(hx[:, :wdt], hT[:, sl])
                ht = tp.tile([P, 512], bf16, name="htc", tag="ewhl")
                ct = tp.tile([P, 512], f32, name="ctc", tag="ewcl")
                nc.sync.dma_start(ht[:, :wdt], HT_in[:, sl])
                nc.sync.dma_start(ct[:, :wdt], CT_in[:, sl])
                pg = [pbig.tile([P, 512], f32, name=f"pl{g}", tag="big")
                      for g in range(4)]
                for g in range(4):
                    gs = slice(g * P, (g + 1) * P)
                    nc.tensor.matmul(pg[g][:, :wdt], lhsT=lWih[:, gs],
                                     rhs=hx[:, :wdt], start=True,
                                     stop=False)
                    nc.tensor.matmul(pg[g][:, :wdt], lhsT=lWhh[:, gs],
                                     rhs=ht[:, :wdt], start=False,
                                     stop=True)
                it = tp.tile([P, 512], f32, name="it", tag="ew1")
                ft = tp.tile([P, 512], f32, name="ft", tag="ew2")
                gt = tp.tile([P, 512], f32, name="gt", tag="ew3")
                ot = tp.tile([P, 512], f32, name="ot", tag="ew4")
                nc.scalar.activation(it[:, :wdt], pg[0][:, :wdt],
                                     AF.Sigmoid, bias=lstmb[:, 0:1])
                nc.scalar.activation(ft[:, :wdt], pg[1][:, :wdt],
                                     AF.Sigmoid, bias=lstmb[:, 1:2])
                nc.scalar.activation(gt[:, :wdt], pg[2][:, :wdt],
                                     AF.Tanh, bias=lstmb[:, 2:3])
                nc.scalar.activation(ot[:, :wdt], pg[3][:, :wdt],
                                     AF.Sigmoid, bias=lstmb[:, 3:4])
                t1 = tp.tile([P, 512], f32, name="lt1", tag="ew5")
                t2 = tp.tile([P, 512], f32, name="lt2", tag="ew6")
                cn = tp.tile([P, 512], f32, name="cn", tag="ew7")
                tc_ = tp.tile([P, 512], f32, name="tcx", tag="ewt")
                hn = tp.tile([P, 512], f32, name="hn", tag="ewh")
                nc.vector.tensor_mul(t1[:, :wdt], ft[:, :wdt], ct[:, :wdt])
                nc.vector.tensor_mul(t2[:, :wdt], it[:, :wdt], gt[:, :wdt])
                nc.vector.tensor_add(cn[:, :wdt], t1[:, :wdt], t2[:, :wdt])
                nc.scalar.activation(tc_[:, :wdt], cn[:, :wdt], AF.Tanh)
                nc.vector.tensor_mul(hn[:, :wdt], ot[:, :wdt], tc_[:, :wdt])
                nc.sync.dma_start(Cout_ext[:, sl], cn[:, :wdt])
                nc.sync.dma_start(Hout_ext[:, sl], hn[:, :wdt])
    return nc


_CACHE = {}


def kernel(X, edge_index, edge_weight, H, C, conv_W,
           gru_Wih, gru_Whh, gru_bih, gru_bhh,
           lstm_Wih, lstm_Whh, lstm_bih, lstm_bhh):
    X = np.asarray(X, dtype=np.float32)
    H = np.asarray(H, dtype=np.float32)
    C = np.asarray(C, dtype=np.float32)
    conv_W = np.asarray(conv_W, dtype=np.float32)
    edge_index = np.asarray(edge_index)
    edge_weight = np.asarray(edge_weight, dtype=np.float32)

    N, D = X.shape
    L = conv_W.shape[0]
    assert D == P and N % (NCORES * NWIN) == 0
    NL = N // NCORES
    NB = (NL + P - 1) // P
    NLP = NB * P
    assert N // NWIN <= 32767

    src = edge_index[0].astype(np.int64)
    dst = edge_index[1].astype(np.int64)
    newpos = _balance_nodes(dst, N, NL, NB)
    perm = np.empty(N, dtype=np.int64)          # new id -> orig id
    perm[newpos] = np.arange(N)
    e_new = np.stack([newpos[src], newpos[dst]])

    edata, cap, pos, ncols, wstart = _preprocess_edges(
        e_new, edge_weight, N, NL, NB)

    # gather calls: GK-chunk ranges, window-pure
    calls = []
    for w in range(NWIN):
        c = wstart[w]
        while c < wstart[w + 1]:
            calls.append((c, min(c + GK, wstart[w + 1])))
            c = calls[-1][1]

    key = (N, D, L, ncols, tuple(cap.ravel()))
    if key not in _CACHE:
        nc = _build(N, D, L, NL, NB, NLP, cap, pos, ncols, wstart, calls)
        nc.compile()
        _CACHE[key] = nc
    nc = _CACHE[key]

    Xp, Hp, Cp = X[perm], H[perm], C[perm]

    gWihT = np.ascontiguousarray(
        np.asarray(gru_Wih, np.float32).T).astype(BF)
    gWhhT = np.ascontiguousarray(
        np.asarray(gru_Whh, np.float32).T).astype(BF)
    lWihT = np.ascontiguousarray(
        np.asarray(lstm_Wih, np.float32).T).astype(BF)
    lWhhT = np.ascontiguousarray(
        np.asarray(lstm_Whh, np.float32).T).astype(BF)
    gb = np.asarray(gru_bih, np.float32)
    gb2 = np.asarray(gru_bhh, np.float32)
    grub = np.stack([gb[0:D] + gb2[0:D], gb[D:2 * D] + gb2[D:2 * D],
                     gb[2 * D:3 * D], gb2[2 * D:3 * D]], axis=1)
    lb = np.asarray(lstm_bih, np.float32) + np.asarray(lstm_bhh, np.float32)
    lstmb = np.stack([lb[g * D:(g + 1) * D] for g in range(4)], axis=1)
    convWb = np.ascontiguousarray(
        np.concatenate([conv_W[i] for i in range(L)], axis=1))

    in_maps = []
    for r in range(NCORES):
        sl = slice(r * NL, (r + 1) * NL)
        in_maps.append(dict(
            hT0=_padT(Xp[sl], NLP),
            HT=_padT(Hp[sl], NLP, BF),
            CT=_padT(Cp[sl], NLP),
            convW=convWb, gWihT=gWihT, gWhhT=gWhhT, grub=grub,
            lWihT=lWihT, lWhhT=lWhhT, lstmb=lstmb,
            esrc16=_pack_idx16(edata[r]['rows'], calls),
            S=edata[r]['S'],
        ))

    if os.environ.get("KERNEL_SIM"):
        from concourse import bass_interp
        sim = bass_interp.MultiCoreSim(nc, NCORES)
        for r in range(NCORES):
            for k, v in in_maps[r].items():
                sim.cores[r].tensor(k)[:] = v
        sim.simulate()
        results = [{k: np.asarray(sim.cores[r].mem_tensor(k))
                    for k in ("HoutT", "CoutT")} for r in range(NCORES)]
    else:
        trace = bool(int(os.environ.get("KERNEL_TRACE", "0")))
        res = run_bass_kernel_spmd(nc, in_maps, core_ids=list(range(NCORES)),
                                   trace=trace)
        if trace:
            kernel.last_exec_time_ns = res.exec_time_ns
        results = res.results

    Hnew = np.empty((N, D), dtype=np.float32)
    Cnew = np.empty((N, D), dtype=np.float32)
    for r in range(NCORES):
        sl = slice(r * NL, (r + 1) * NL)
        Hnew[sl] = results[r]["HoutT"].T[:NL]
        Cnew[sl] = results[r]["CoutT"].T[:NL]
    Hout = Hnew[newpos]
    Cout = Cnew[newpos]
    return Hout, Hout, Cout


kernel.last_exec_time_ns = None


# revision 14
# speedup vs baseline: 1.3141x; 1.2749x over previous
"""DyGrEncoder (GatedGraphConv x3 + GRUCell + LSTM) as a Bass/Tile SPMD kernel
on 8 TRN2 NeuronCores.

Sharding: nodes row-wise across 8 cores with a degree-balanced permutation.
Per conv layer: local m = h @ W (fp32), AllGather m (bf16) via DRAM bounce,
then a batched dma_gather (custom SWDGE gather, int16 indices) pulls per-edge
source rows from m_full in 4 source-windows of N/4 rows; the weighted
segment-sum is one-hot matmuls (host-precomputed S, streamed bf16) into
[128,512] PSUM superblock tiles, merged across windows into bf16 aggT.
GRU runs in bf16 matmuls (FWL) with fp32 state; final LSTM is node-parallel.
"""
import os
import numpy as np
import ml_dtypes

import concourse.bass as bass
import concourse.mybir as mybir
import concourse.tile as tile
from concourse import bacc, library_config
from concourse.bass_utils import run_bass_kernel_spmd

P = 128
NCORES = 8
NWIN = 4     # source windows (int16 gather index limit)
GK = 16      # edge-chunks per dma_gather call
f32 = mybir.dt.float32
bf16 = mybir.dt.bfloat16
i32 = mybir.dt.int32
i16 = mybir.dt.int16
AF = mybir.ActivationFunctionType
ALU = mybir.AluOpType
BF = ml_dtypes.bfloat16


# ----------------------------------------------------------------- host side

def _win_blocks(NB):
    """Split NB blocks into NWIN contiguous groups (source windows)."""
    base = NB // NWIN
    rem = NB % NWIN
    sizes = [base + (1 if w < rem else 0) for w in range(NWIN)]
    wb = [0]
    for s in sizes:
        wb.append(wb[-1] + s)
    return wb


def _balance_nodes(dst, N, NL, NB):
    """Permute nodes so each of the 8*NB destination blocks holds 128 nodes
    whose total in-degree sits just under a multiple of 128. Returns newpos
    (orig id -> new id); new id = (core r, block j, slot) = r*NL + j*128 + s.
    """
    indeg = np.bincount(dst, minlength=N).astype(np.int64)
    order = np.argsort(-indeg, kind='stable')      # high degree first
    lastw = NL - (NB - 1) * P                      # slots in last position
    tail_n = lastw * NCORES                        # lowest-degree nodes there
    NBF = NB - 1                                   # full positions
    body = order[:N - tail_n]
    tail = order[N - tail_n:]
    E_body = int(indeg[body].sum())
    total_chunks = (E_body + 127) // 128

    q = total_chunks // (NBF * NCORES)             # per-block chunks target
    n_high = 0
    margin = 10
    sorted_deg = indeg[body]
    csum = np.concatenate([[0], np.cumsum(sorted_deg)])
    NBODY = len(body)
    while True:
        hi_bins = n_high * NCORES
        lo_bins = (NBF - n_high) * NCORES
        hi_nodes = hi_bins * P
        ok = True
        if hi_bins:
            t_hi = csum[hi_nodes]
            if t_hi / hi_bins > (q + 1) * P - margin:
                ok = False
        if lo_bins:
            t_lo = csum[NBODY] - csum[hi_nodes]
            if t_lo / lo_bins > q * P - margin:
                ok = False
        if ok or n_high >= NBF:
            break
        n_high += 1

    def snake(ids, nbins):
        k = len(ids) // nbins
        bins = [[] for _ in range(nbins)]
        pos = 0
        for rnd in range(k):
            idxs = range(nbins) if rnd % 2 == 0 else range(nbins - 1, -1, -1)
            for b in idxs:
                bins[b].append(ids[pos])
                pos += 1
        return bins

    hi_bins_n = n_high * NCORES
    hi_ids = body[:hi_bins_n * P]
    lo_ids = body[hi_bins_n * P:]
    bins = []
    if hi_bins_n:
        bins += snake(hi_ids, hi_bins_n)
    if NBF - n_high:
        bins += snake(lo_ids, (NBF - n_high) * NCORES)
    bins += snake(tail, NCORES)

    newpos = np.empty(N, dtype=np.int64)
    bi = 0
    for j in range(NB):
        for r in range(NCORES):
            ids = np.array(bins[bi])
            base = r * NL + j * P
            newpos[ids] = base + np.arange(len(ids))
            bi += 1
    return newpos


def _preprocess_edges(edge_index, edge_weight, N, NL, NB):
    """Per core: incoming edges binned by (dest block j, source window w),
    sorted by source row inside each bin, padded to cap[j][w]*128 edges
    (caps shared across cores). Chunk order is window-major:
    (w, superblock, j, k). Returns per-core packed arrays + structure.

    Source windows are block-aligned quarters of each core's local shard:
    window w of the AllGather output m_full_w holds, for every core r, its
    local blocks [wb[w], wb[w+1]); a source node (r, o) lives at row
    r*wrows[w] + (o - wb[w]*128) of m_full_w.
    """
    wb = _win_blocks(NB)
    wrows = [min(wb[w + 1] * P, NL) - wb[w] * P for w in range(NWIN)]
    src = np.asarray(edge_index[0]).astype(np.int64)
    dst = np.asarray(edge_index[1]).astype(np.int64)
    w_all = np.asarray(edge_weight).astype(np.float32)

    s_r = src // NL
    s_o = src % NL
    s_t = s_o // P
    s_w = np.digitize(s_t, wb[1:NWIN])              # window of source block
    wrows_arr = np.asarray(wrows)
    wb_arr = np.asarray([wb[w] * P for w in range(NWIN)])
    s_row = s_r * wrows_arr[s_w] + (s_o - wb_arr[s_w])

    per_core = []
    counts = np.zeros((NCORES, NB, NWIN), dtype=np.int64)
    for r in range(NCORES):
        lo, hi = r * NL, (r + 1) * NL
        m = (dst >= lo) & (dst < hi)
        es, ed, ew, wv = s_row[m], dst[m] - lo, w_all[m], s_w[m]
        order = np.lexsort((es, wv, ed // P))
        es, ed, ew, wv = es[order], ed[order], ew[order], wv[order]
        np.add.at(counts[r], (ed // P, wv), 1)
        per_core.append((es, ed, ew, wv))

    cap = np.maximum(np.ceil(counts / P).astype(np.int64).max(axis=0), 1)

    # global chunk positions, window-major over superblocks
    NSB = (NB + 3) // 4
    pos = np.zeros((NB, NWIN), dtype=np.int64)
    c = 0
    wstart = []
    for w in range(NWIN):
        wstart.append(c)
        for sb in range(NSB):
            for j in range(sb * 4, min(sb * 4 + 4, NB)):
                pos[j, w] = c
                c += int(cap[j, w])
    ncols = c
    wstart.append(c)

    out = []
    for r in range(NCORES):
        es, ed, ew, wv = per_core[r]
        rows = np.zeros((ncols, P), dtype=np.int32)   # window-relative rows
        slot = np.zeros((ncols, P), dtype=np.int64)
        wgt = np.zeros((ncols, P), dtype=np.float32)
        # bin starts within the sorted arrays
        start = 0
        for j in range(NB):
            for w in range(NWIN):
                cnt = int(counts[r, j, w])
                seg = slice(start, start + cnt)
                c0 = int(pos[j, w])
                ch = np.arange(cnt) // P + c0
                sl = np.arange(cnt) % P
                rows[ch, sl] = es[seg].astype(np.int32)
                slot[ch, sl] = ed[seg] - j * P
                wgt[ch, sl] = ew[seg]
                start += cnt
        # S: [128 edge-slot, ncols*128], S[e, c*128+s] = w at dest slot s
        S3 = np.zeros((ncols, P, P), dtype=np.float32)
        np.put_along_axis(S3, slot[:, :, None], wgt[:, :, None], axis=2)
        S_full = np.ascontiguousarray(
            S3.transpose(1, 0, 2).reshape(P, ncols * P)).astype(BF)
        out.append(dict(rows=rows, S=S_full))
    return out, cap, pos, ncols, wstart


def _pack_idx16(rows, calls):
    """rows [ncols, 128] int32 -> [128, ncols*8] int16 in the dma_gather wrap
    layout, per call: flat q = (c-c0)*128 + p -> [q%16, c0*8 + q//16]."""
    ncols = rows.shape[0]
    out = np.zeros((16, ncols * 8), dtype=np.int16)
    for (c0, c1) in calls:
        f = rows[c0:c1].reshape(-1).astype(np.int16)
        q = np.arange(len(f))
        out[q % 16, c0 * 8 + q // 16] = f
    return np.tile(out, (8, 1))


def _padT(a, NLP, dt=np.float32):
    aT = np.ascontiguousarray(np.asarray(a).T.astype(np.float32))
    out = np.zeros((aT.shape[0], NLP), dtype=np.float32)
    out[:, :aT.shape[1]] = aT
    return out.astype(dt)


# ---------------------------------------------------------------- bass build

def _build(N, D, L, NL, NB, NLP, cap, pos, ncols, wstart, calls):
    nc = bacc.Bacc("TRN2", target_bir_lowering=False, debug=False,
                   num_devices=NCORES, num_swdge_queues=4)
    dp = nc.declare_dram_parameter
    wb = _win_blocks(NB)
    wrows = [min(wb[w + 1] * P, NL) - wb[w] * P for w in range(NWIN)]

    hT0_in = dp("hT0", [P, NLP], f32, isOutput=False)
    HT_in = dp("HT", [P, NLP], bf16, isOutput=False)
    CT_in = dp("CT", [P, NLP], f32, isOutput=False)
    convW_in = dp("convW", [P, L * P], f32, isOutput=False)
    gWih_in = dp("gWihT", [P, 3 * P], bf16, isOutput=False)
    gWhh_in = dp("gWhhT", [P, 3 * P], bf16, isOutput=False)
    grub_in = dp("grub", [P, 4], f32, isOutput=False)
    lWih_in = dp("lWihT", [P, 4 * P], bf16, isOutput=False)
    lWhh_in = dp("lWhhT", [P, 4 * P], bf16, isOutput=False)
    lstmb_in = dp("lstmb", [P, 4], f32, isOutput=False)
    esrc_in = dp("esrc16", [P, ncols * 8], i16, isOutput=False)
    S_in = dp("S", [P, ncols * P], bf16, isOutput=False)
    Hout_ext = dp("HoutT", [P, NLP], f32, isOutput=True)
    Cout_ext = dp("CoutT", [P, NLP], f32, isOutput=True)

    lastw = NL - (NB - 1) * P
    chunks = [(s, min(512, NLP - s)) for s in range(0, NLP, 512)]
    NSB = (NB + 3) // 4

    with tile.TileContext(nc) as tc:
        with (
            tc.tile_pool(name="dram", bufs=1, space="DRAM") as dram,
            tc.tile_pool(name="persist", bufs=1) as pers,
            tc.tile_pool(name="msgp", bufs=3) as msgp,
            tc.tile_pool(name="sp", bufs=3) as sp,
            tc.tile_pool(name="tmp", bufs=2) as tp,
            tc.tile_pool(name="pagg", bufs=2, space="PSUM") as pagg,
            tc.tile_pool(name="pbig", bufs=4, space="PSUM") as pbig,
        ):
            nc.gpsimd.load_library(library_config.mlp)

            hT = pers.tile([P, NLP], f32, name="hT")
            convW = pers.tile([P, L * P], f32, name="convW")
            gWih = pers.tile([P, 3 * P], bf16, name="gWih")
            gWhh = pers.tile([P, 3 * P], bf16, name="gWhh")
            grub = pers.tile([P, 4], f32, name="grub")
            lWih = pers.tile([P, 4 * P], bf16, name="lWih")
            lWhh = pers.tile([P, 4 * P], bf16, name="lWhh")
            lstmb = pers.tile([P, 4], f32, name="lstmb")
            esrc = pers.tile([P, ncols * 8], i16, name="esrc")

            nc.sync.dma_start(hT[:], hT0_in[:])
            nc.sync.dma_start(convW[:], convW_in[:])
            nc.sync.dma_start(gWih[:], gWih_in[:])
            nc.sync.dma_start(gWhh[:], gWhh_in[:])
            nc.sync.dma_start(grub[:], grub_in[:])
            nc.sync.dma_start(lWih[:], lWih_in[:])
            nc.sync.dma_start(lWhh[:], lWhh_in[:])
            nc.sync.dma_start(lstmb[:], lstmb_in[:])
            nc.sync.dma_start(esrc[:], esrc_in[:])

            with tc.tile_pool(name="conv", bufs=1) as convp:
                aggT = convp.tile([P, NLP], bf16, name="aggT")
                m_sb = convp.tile([P, NLP], bf16, name="m_sb")

                for l in range(L):
                    # ---- 1. m_local = h @ W[l]  (node-major, bf16 out)
                    for t0 in range(0, NB, 4):
                        te = min(t0 + 4, NB)
                        pm = pagg.tile([P, 512], f32, name="pm", tag="pk")
                        for t in range(t0, te):
                            o = (t - t0) * P
                            nc.tensor.matmul(
                                pm[:, o:o + P],
                                lhsT=hT[:, t * P:(t + 1) * P],
                                rhs=convW[:, l * P:(l + 1) * P],
                                start=True, stop=True)
                        nc.scalar.copy(out=m_sb[:, t0 * P:te * P],
                                       in_=pm[:, :(te - t0) * P])

                    # ---- 2. bounce (node-major [NL, D]) + per-window AGs
                    m_bounce = dram.tile([NL, P], bf16, name=f"mb{l}")
                    m_fulls = []
                    m3 = m_sb[:].rearrange("p (t f) -> p t f", f=P)
                    for w in range(NWIN):
                        t0, t1 = wb[w], wb[w + 1]
                        r0 = t0 * P
                        mfw = dram.tile([NCORES * wrows[w], P], bf16,
                                        name=f"mf{l}_{w}",
                                        addr_space="Shared")
                        m_fulls.append(mfw)
                        tf = t1 - 1 if t1 == NB else t1   # full blocks end
                        if tf > t0:
                            nc.sync.dma_start(
                                m_bounce[r0:tf * P, :].rearrange(
                                    "(t p) f -> p t f", p=P),
                                m3[:, t0:tf, :])
                        if t1 == NB:
                            nc.sync.dma_start(
                                m_bounce[(NB - 1) * P:, :],
                                m3[:lastw, NB - 1, :])
                        nc.gpsimd.collective_compute(
                            "AllGather", ALU.bypass,
                            replica_groups=[list(range(NCORES))],
                            ins=[m_bounce[r0:r0 + wrows[w], :].opt()],
                            outs=[mfw[:].opt()])

                    # ---- 3. batched gathers + S stream (window-major) and
                    #         one-hot matmul segment-sum into aggT; GRU for
                    #         superblock sb interleaved right after its last
                    #         window merge so it pipelines inside the drain.
                    ctile = {}   # global chunk -> (msg tile, S tile, col)
                    for qi, (c0, c1) in enumerate(calls):
                        kk = c1 - c0
                        msgb = msgp.tile([P, GK, P], bf16, name="msgb",
                                         tag="msgb")
                        Sb = sp.tile([P, GK * P], bf16, name="Sb", tag="Sb")
                        w = 0
                        while not (wstart[w] <= c0 < wstart[w + 1]):
                            w += 1
                        nc.sync.dma_start(Sb[:, :kk * P],
                                          S_in[:, c0 * P:c1 * P])
                        nc.gpsimd.dma_gather(
                            out_ap=msgb[:, :kk, :],
                            in_ap=m_fulls[w][:],
                            idxs_ap=esrc[:, c0 * 8:c1 * 8],
                            num_idxs=kk * P, num_idxs_reg=kk * P,
                            elem_size=P, single_packet=False,
                            queue_num=qi % 4)
                        for c in range(c0, c1):
                            ctile[c] = (msgb, Sb, c - c0)

                    for w in range(NWIN):
                        for sb in range(NSB):
                            j0, je = sb * 4, min(sb * 4 + 4, NB)
                            pj = pagg.tile([P, 512], f32, name="pj", tag="pk")
                            for j in range(j0, je):
                                kj = int(cap[j, w])
                                base = int(pos[j, w])
                                for k in range(kj):
                                    mt, St, col = ctile[base + k]
                                    nc.tensor.matmul(
                                        pj[:, (j - j0) * P:(j - j0 + 1) * P],
                                        lhsT=mt[:, col, :],
                                        rhs=St[:, col * P:(col + 1) * P],
                                        start=(k == 0), stop=(k == kj - 1))
                            sl = slice(j0 * P, je * P)
                            wdt = (je - j0) * P
                            if w == 0:
                                nc.scalar.copy(out=aggT[:, sl],
                                               in_=pj[:, :wdt])
                            else:
                                nc.vector.tensor_add(aggT[:, sl],
                                                     aggT[:, sl],
                                                     pj[:, :wdt])
                            if w != NWIN - 1:
                                continue

                            # ---- 4. GRU for this superblock (x = aggT bf16)
                            s = sb * 512
                            wdt = min(512, NLP - s)
                            sl = slice(s, s + wdt)
                            hb = tp.tile([P, 512], bf16, name="hb",
                                         tag="ewhb")
                            nc.vector.tensor_copy(hb[:, :wdt], hT[:, sl])
                            pr = pbig.tile([P, 512], f32, name="pr",
                                           tag="big")
                            pz = pbig.tile([P, 512], f32, name="pz",
                                           tag="big")
                            pin = pbig.tile([P, 512], f32, name="pin",
                                            tag="big")
                            phn = pbig.tile([P, 512], f32, name="phn",
                                            tag="big")
                            for (ps, g) in ((pr, 0), (pz, 1)):
                                gs = slice(g * P, (g + 1) * P)
                                nc.tensor.matmul(ps[:, :wdt],
                                                 lhsT=gWih[:, gs],
                                                 rhs=aggT[:, sl],
                                                 start=True, stop=False)
                                nc.tensor.matmul(ps[:, :wdt],
                                                 lhsT=gWhh[:, gs],
                                                 rhs=hb[:, :wdt],
                                                 start=False, stop=True)
                            gn = slice(2 * P, 3 * P)
                            nc.tensor.matmul(pin[:, :wdt], lhsT=gWih[:, gn],
                                             rhs=aggT[:, sl],
                                             start=True, stop=True)
                            nc.tensor.matmul(phn[:, :wdt], lhsT=gWhh[:, gn],
                                             rhs=hb[:, :wdt],
                                             start=True, stop=True)

                            rt = tp.tile([P, 512], f32, name="rt", tag="ew1")
                            zt = tp.tile([P, 512], f32, name="zt", tag="ew2")
                            t2 = tp.tile([P, 512], f32, name="t2", tag="ew3")
                            t3 = tp.tile([P, 512], f32, name="t3", tag="ew4")
                            nt = tp.tile([P, 512], f32, name="nt", tag="ew5")
                            dd = tp.tile([P, 512], f32, name="dd", tag="ew6")
                            ee = tp.tile([P, 512], f32, name="ee", tag="ew7")
                            nc.scalar.activation(rt[:, :wdt], pr[:, :wdt],
                                                 AF.Sigmoid,
                                                 bias=grub[:, 0:1])
                            nc.scalar.activation(zt[:, :wdt], pz[:, :wdt],
                                                 AF.Sigmoid,
                                                 bias=grub[:, 1:2])
                            nc.vector.scalar_tensor_tensor(
                                out=t2[:, :wdt], in0=phn[:, :wdt],
                                scalar=grub[:, 3:4], in1=rt[:, :wdt],
                                op0=ALU.add, op1=ALU.mult)
                            nc.vector.tensor_add(t3[:, :wdt], t2[:, :wdt],
                                                 pin[:, :wdt])
                            nc.scalar.activation(nt[:, :wdt], t3[:, :wdt],
                                                 AF.Tanh, bias=grub[:, 2:3])
                            nc.vector.tensor_sub(dd[:, :wdt], hT[:, sl],
                                                 nt[:, :wdt])
                            nc.vector.tensor_mul(ee[:, :wdt], zt[:, :wdt],
                                                 dd[:, :wdt])
                            nc.vector.tensor_add(hT[:, sl], nt[:, :wdt],
                                                 ee[:, :wdt])

            # ---- LSTM (x = hT cast bf16, hidden bf16, cell fp32)
            for (s, wdt) in chunks:
                sl = slice(s, s + wdt)
                hx = tp.tile([P, 512], bf16, name="hx", tag="ewhb")
                nc.vector.tensor_copy(hx[:, :wdt], hT[:, sl])
                ht = tp.tile([P, 512], bf16, name="htc", tag="ewhl")
                ct = tp.tile([P, 512], f32, name="ctc", tag="ewcl")
                nc.sync.dma_start(ht[:, :wdt], HT_in[:, sl])
                nc.sync.dma_start(ct[:, :wdt], CT_in[:, sl])
                pg = [pbig.tile([P, 512], f32, name=f"pl{g}", tag="big")
                      for g in range(4)]
                for g in range(4):
                    gs = slice(g * P, (g + 1) * P)
                    nc.tensor.matmul(pg[g][:, :wdt], lhsT=lWih[:, gs],
                                     rhs=hx[:, :wdt], start=True,
                                     stop=False)
                    nc.tensor.matmul(pg[g][:, :wdt], lhsT=lWhh[:, gs],
                                     rhs=ht[:, :wdt], start=False,
                                     stop=True)
                it = tp.tile([P, 512], f32, name="it", tag="ew1")
                ft = tp.tile([P, 512], f32, name="ft", tag="ew2")
                gt = tp.tile([P, 512], f32, name="gt", tag="ew3")
                ot = tp.tile([P, 512], f32, name="ot", tag="ew4")
                nc.scalar.activation(it[:, :wdt], pg[0][:, :wdt],
                                     AF.Sigmoid, bias=lstmb[:, 0:1])
                nc.scalar.activation(ft[:, :wdt], pg[1][:, :wdt],
                                     AF.Sigmoid, bias=lstmb[:, 1:2])
                nc.scalar.activation(gt[:, :wdt], pg[2][:, :wdt],
                                     AF.Tanh, bias=lstmb[:, 2:3])
                nc.scalar.activation(ot[:, :wdt], pg[3][:, :wdt],
                                     AF.Sigmoid, bias=lstmb[:, 3:4])
                t1 = tp.tile([P, 512], f32, name="lt1", tag="ew5")
                t2 = tp.tile([P, 512], f32, name="lt2", tag="ew6")
                cn = tp.tile([P, 512], f32, name="cn", tag="ew7")
                tc_ = tp.tile([P, 512], f32, name="tcx", tag="ewt")
                hn = tp.tile([P, 512], f32, name="hn", tag="ewh")
                nc.vector.tensor_mul(t1[:, :wdt], ft[:, :wdt], ct[:, :wdt])
                nc.vector.tensor_mul(t2[:, :wdt], it[:, :wdt], gt[:, :wdt])
                nc.vector.tensor_add(cn[:, :wdt], t1[:, :wdt], t2[:, :wdt])
                nc.scalar.activation(tc_[:, :wdt], cn[:, :wdt], AF.Tanh)
                nc.vector.tensor_mul(hn[:, :wdt], ot[:, :wdt], tc_[:, :wdt])
                nc.sync.dma_start(Cout_ext[:, sl], cn[:, :wdt])
                nc.sync.dma_start(Hout_ext[:, sl], hn[:, :wdt])
    return nc


_CACHE = {}


def kernel(X, edge_index, edge_weight, H, C, conv_W,
           gru_Wih, gru_Whh, gru_bih, gru_bhh,
           lstm_Wih, lstm_Whh, lstm_bih, lstm_bhh):
    X = np.asarray(X, dtype=np.float32)
    H = np.asarray(H, dtype=np.float32)
    C = np.asarray(C, dtype=np.float32)
    conv_W = np.asarray(conv_W, dtype=np.float32)
    edge_index = np.asarray(edge_index)
    edge_weight = np.asarray(edge_weight, dtype=np.float32)

    N, D = X.shape
    L = conv_W.shape[0]
    assert D == P and N % NCORES == 0
    NL = N // NCORES
    NB = (NL + P - 1) // P
    NLP = NB * P
    wb = _win_blocks(NB)
    assert max(wb[w + 1] - wb[w] for w in range(NWIN)) * P * NCORES <= 32767

    src = edge_index[0].astype(np.int64)
    dst = edge_index[1].astype(np.int64)
    newpos = _balance_nodes(dst, N, NL, NB)
    perm = np.empty(N, dtype=np.int64)          # new id -> orig id
    perm[newpos] = np.arange(N)
    e_new = np.stack([newpos[src], newpos[dst]])

    edata, cap, pos, ncols, wstart = _preprocess_edges(
        e_new, edge_weight, N, NL, NB)

    # gather calls: GK-chunk ranges, window-pure
    calls = []
    for w in range(NWIN):
        c = wstart[w]
        while c < wstart[w + 1]:
            calls.append((c, min(c + GK, wstart[w + 1])))
            c = calls[-1][1]

    key = (N, D, L, ncols, tuple(cap.ravel()))
    if key not in _CACHE:
        nc = _build(N, D, L, NL, NB, NLP, cap, pos, ncols, wstart, calls)
        nc.compile()
        _CACHE[key] = nc
    nc = _CACHE[key]

    Xp, Hp, Cp = X[perm], H[perm], C[perm]

    gWihT = np.ascontiguousarray(
        np.asarray(gru_Wih, np.float32).T).astype(BF)
    gWhhT = np.ascontiguousarray(
        np.asarray(gru_Whh, np.float32).T).astype(BF)
    lWihT = np.ascontiguousarray(
        np.asarray(lstm_Wih, np.float32).T).astype(BF)
    lWhhT = np.ascontiguousarray(
        np.asarray(lstm_Whh, np.float32).T).astype(BF)
    gb = np.asarray(gru_bih, np.float32)
    gb2 = np.asarray(gru_bhh, np.float32)
    grub = np.stack([gb[0:D] + gb2[0:D], gb[D:2 * D] + gb2[D:2 * D],
                     gb[2 * D:3 * D], gb2[2 * D:3 * D]], axis=1)
    lb = np.asarray(lstm_bih, np.float32) + np.asarray(lstm_bhh, np.float32)
    lstmb = np.stack([lb[g * D:(g + 1) * D] for g in range(4)], axis=1)
    convWb = np.ascontiguousarray(
        np.concatenate([conv_W[i] for i in range(L)], axis=1))

    in_maps = []
    for r in range(NCORES):
        sl = slice(r * NL, (r + 1) * NL)
        in_maps.append(dict(
            hT0=_padT(Xp[sl], NLP),
            HT=_padT(Hp[sl], NLP, BF),
            CT=_padT(Cp[sl], NLP),
            convW=convWb, gWihT=gWihT, gWhhT=gWhhT, grub=grub,
            lWihT=lWihT, lWhhT=lWhhT, lstmb=lstmb,
            esrc16=_pack_idx16(edata[r]['rows'], calls),
            S=edata[r]['S'],
        ))

    if os.environ.get("KERNEL_SIM"):
        from concourse import bass_interp
        sim = bass_interp.MultiCoreSim(nc, NCORES)
        for r in range(NCORES):
            for k, v in in_maps[r].items():
                sim.cores[r].tensor(k)[:] = v
        sim.simulate()
        results = [{k: np.asarray(sim.cores[r].mem_tensor(k))
                    for k in ("HoutT", "CoutT")} for r in range(NCORES)]
    else:
        trace = bool(int(os.environ.get("KERNEL_TRACE", "0")))
        res = run_bass_kernel_spmd(nc, in_maps, core_ids=list(range(NCORES)),
                                   trace=trace)
        if trace:
            kernel.last_exec_time_ns = res.exec_time_ns
        results = res.results

    Hnew = np.empty((N, D), dtype=np.float32)
    Cnew = np.empty((N, D), dtype=np.float32)
    for r in range(NCORES):
        sl = slice(r * NL, (r + 1) * NL)
        Hnew[sl] = results[r]["HoutT"].T[:NL]
        Cnew[sl] = results[r]["CoutT"].T[:NL]
    Hout = Hnew[newpos]
    Cout = Cnew[newpos]
    return Hout, Hout, Cout


kernel.last_exec_time_ns = None
